# revision 1
# baseline (speedup 1.0000x reference)
"""AncProbsLayer Trainium2 kernel (8 NeuronCores, data-parallel over batch).

out[b,l,h,0,z] = sum_d seq[b,l,0,0,d] * P[b,h,d,z]
P[b,h] = diag(1/sqrt_pi_h) V_h diag(exp(lam_h * tau[b,h])) V_h^T diag(sqrt_pi_h)

The GTR eigendecomposition (H=8 independent symmetric 20x20 eigh) is
precomputed on host (the reference marks it "constant (non-trainable)");
the per-(b,h) transition matrices P are assembled on host (tiny: 1.6 MB)
and the device does the memory-bound part: the (B*L, 20) x (20, 160)
batched matmul producing the 160 MiB output.

Device structure (per core: BS=16 batches, paired):
- batches processed in PAIRS: lhsT stacks both batches' 20 d-rows (K=40),
  rhs is the block-diagonal [[W_b0, 0], [0, W_b1]] (40, 320), so one
  LDWEIGHTS+MATMUL computes a 128-row l-tile for BOTH batches (N=320
  fits one PSUM bank). Halves the LDWEIGHTS-paced PE time.
- l rows are interleaved on host so PSUM partition p, tile t holds row
  16p+t: each SBUF partition's 16 tiles form one contiguous 10KB run in
  the (paired-layout) output -> line-rate DMA descriptors.
- PSUM -> SBUF copies (mandatory: DMA can't read PSUM) are split between
  the Vector and Scalar engines, casting f32 -> bf16; the bf16 output is
  upcast to f32 on host (rel-err ~2e-3, well under the 2e-2 gate).
"""

import sys

sys.path.insert(0, "/opt/trn_rl_repo")
sys.path.insert(0, "/root/.axon_site")

import numpy as np


def _install_axon_hooks_shim():
    """The agent image's antenv lacks axon_hooks; bass_utils imports it when
    BASS_TRACE=1. Provide it (registering the ctypes NTFF hook if possible)
    so tracing degrades gracefully instead of crashing."""
    try:
        import antenv.axon_hooks  # noqa: F401

        return
    except ImportError:
        pass
    try:
        import types

        mod = types.ModuleType("antenv.axon_hooks")
        _h = [None]
        mod.set_axon_ntff_profile_hook = lambda h: _h.__setitem__(0, h)
        mod.get_axon_ntff_profile_hook = lambda: _h[0]
        sys.modules["antenv.axon_hooks"] = mod
        import antenv

        antenv.axon_hooks = mod
        try:
            from trn_agent_boot.trn_boot import _ntff_profile_via_ctypes

            mod.set_axon_ntff_profile_hook(
                _ntff_profile_via_ctypes("/opt/axon/libaxon_pjrt.so")
            )
        except Exception:
            pass
    except Exception:
        pass


_install_axon_hooks_shim()

B, L, H, D = 128, 2048, 8, 20
N_CORES = 8
BS = B // N_CORES  # batches per core
PAIRS = BS // 2
HZ = H * D  # 160 output cols per (b, l)
LT = L // 128  # l-tiles per batch

_NC = None  # compiled Bass graph cache
LAST_RESULTS = None  # BassKernelResults of the most recent run (for profiling)
LAST_IN_MAPS = None  # per-core input shards of the most recent run


def _build_nc():
    import concourse.bacc as bacc
    import concourse.tile as tile
    import concourse.mybir as mybir

    f32 = mybir.dt.float32
    bf16 = mybir.dt.bfloat16
    nc = bacc.Bacc(None, target_bir_lowering=False)

    seqt = nc.declare_dram_parameter("seqt", [2 * D, PAIRS * L], bf16, isOutput=False)
    w = nc.declare_dram_parameter("w", [2 * D, PAIRS * 2 * HZ], bf16, isOutput=False)
    out = nc.declare_dram_parameter("out", [PAIRS * L, 2 * HZ], bf16, isOutput=True)

    with tile.TileContext(nc) as tc:
        with (
            tc.tile_pool(name="wpool", bufs=1) as wpool,
            tc.tile_pool(name="spool", bufs=9) as spool,
            tc.tile_pool(name="psum", bufs=8, space="PSUM") as ppool,
            tc.tile_pool(name="opool", bufs=5) as opool,
        ):
            wt = wpool.tile([2 * D, PAIRS * 2 * HZ], bf16)
            for pr in range(PAIRS):
                nc.sync.dma_start(
                    wt[:, pr * 2 * HZ : (pr + 1) * 2 * HZ],
                    w[:, pr * 2 * HZ : (pr + 1) * 2 * HZ],
                )
            # prefetch ALL seqt during the startup ramp (the DMA engines are
            # otherwise idle there), so the steady state carries pure output
            # traffic on HBM. All tiles stay live (spool holds 9 slots).
            # Pair 0 is split in halves so the first matmul starts sooner.
            seq_tiles = []
            for pr in range(PAIRS):
                if pr == 0:
                    sh = []
                    for j in range(2):
                        stj = spool.tile([2 * D, L // 2], bf16, tag="st")
                        nc.gpsimd.dma_start(
                            stj[:],
                            seqt[:, j * (L // 2) : (j + 1) * (L // 2)],
                        )
                        sh.append(stj)
                    seq_tiles.append(sh)
                else:
                    stp = spool.tile([2 * D, L], bf16, tag="st")
                    nc.gpsimd.dma_start(stp[:], seqt[:, pr * L : (pr + 1) * L])
                    seq_tiles.append([stp])
            for pr in range(PAIRS):
                sh = seq_tiles[pr]
                ot = opool.tile([128, LT, 2 * HZ], bf16)
                for t in range(LT):
                    if pr == 0:
                        st = sh[t // (LT // 2)]
                        tt = t % (LT // 2)
                    else:
                        st = sh[0]
                        tt = t
                    ps = ppool.tile([128, 2 * HZ], f32)
                    nc.tensor.matmul(
                        ps[:],
                        st[:, tt * 128 : (tt + 1) * 128],
                        wt[:, pr * 2 * HZ : (pr + 1) * 2 * HZ],
                        start=True,
                        stop=True,
                    )
                    # PSUM->SBUF cast copies split across both capable engines
                    if (pr * LT + t) % 2 == 0:
                        nc.vector.tensor_copy(ot[:, t, :], ps[:])
                    else:
                        nc.scalar.copy(ot[:, t, :], ps[:])
                # partition p, tile t holds original row l=16p+t of both
                # batches: contiguous 16*320*2B = 10KB per partition.
                dst = out[pr * L : (pr + 1) * L, :].rearrange(
                    "(p t) zz -> p t zz", p=128, t=LT
                )
                # first pair: quarter-split so the output stream starts as
                # early as possible; last pairs: quarter-split so the tail
                # drains with more queue parallelism
                nsplit = 4 if (pr == 0 or pr >= PAIRS - 2) else 2
                chunk = LT // nsplit
                for j in range(nsplit):
                    nc.sync.dma_start(
                        dst[:, j * chunk : (j + 1) * chunk, :],
                        ot[:, j * chunk : (j + 1) * chunk, :],
                    )
    nc.compile()
    return nc


def _get_nc():
    global _NC
    if _NC is None:
        _NC = _build_nc()
    return _NC


def _host_precompute(rate_indices, tau_kernel, exchangeability_kernel, equilibrium_kernel):
    """Everything up to the per-(b,h) 20x20 transition matrices, in float64."""
    ek = exchangeability_kernel.astype(np.float64)[:, 0]  # (H, D, D)
    eq = equilibrium_kernel.astype(np.float64)[:, 0]  # (H, D)

    R = np.logaddexp(ek, 0.0)
    R = 0.5 * (R + R.transpose(0, 2, 1))
    m = eq.max(axis=-1, keepdims=True)
    p = np.exp(eq - m)
    p /= p.sum(axis=-1, keepdims=True)

    Q = R * p[:, None, :]
    diag = Q.sum(axis=-1)
    Q = Q - diag[:, :, None] * np.eye(D)
    mue = (p * diag).sum(axis=-1)
    Q = Q / np.maximum(mue, 1e-16)[:, None, None]

    sq = np.sqrt(p)
    isq = 1.0 / sq
    S = sq[:, :, None] * Q * isq[:, None, :]
    S = 0.5 * (S + S.transpose(0, 2, 1))
    lam, V = np.linalg.eigh(S)  # (H, D), (H, D, D)

    W1 = isq[:, :, None] * V  # (H, D, D): rows d, cols k
    W2 = V.transpose(0, 2, 1) * sq[:, None, :]  # (H, D, D): rows k, cols z

    tau_g = tau_kernel[rate_indices, np.arange(H)[None, :], 0].astype(np.float64)
    tau = np.logaddexp(np.clip(tau_g, -80.0, 80.0), 0.0)  # (B, H)
    e = np.exp(lam[None, :, :] * tau[:, :, None])  # (B, H, D)

    P = np.einsum("hdk,bhk,hkz->bhdz", W1, e, W2)  # (B, H, D, D)
    # w[b][d, h*20+z] = P[b,h,d,z]
    return np.ascontiguousarray(P.transpose(0, 2, 1, 3)).reshape(B, D, HZ).astype(np.float32)


def kernel(sequences, rate_indices, tau_kernel, exchangeability_kernel, equilibrium_kernel):
    global LAST_RESULTS, LAST_IN_MAPS
    from concourse.bass_utils import run_bass_kernel_spmd
    import ml_dtypes

    sequences = np.asarray(sequences)
    rate_indices = np.asarray(rate_indices)
    tau_kernel = np.asarray(tau_kernel)
    exchangeability_kernel = np.asarray(exchangeability_kernel)
    equilibrium_kernel = np.asarray(equilibrium_kernel)

    w_all = _host_precompute(
        rate_indices, tau_kernel, exchangeability_kernel, equilibrium_kernel
    )
    seq = np.asarray(sequences, dtype=np.float32).reshape(B, L, D)

    # interleave l within each batch: device l-tile t, position q <- row 16q+t
    # (so each psum partition's 16 tiles land contiguous in the output)
    seq_il = seq.reshape(B, L // 16, 16, D).transpose(0, 2, 1, 3).reshape(B, L, D)

    in_maps = []
    for c in range(N_CORES):
        b0 = c * BS
        # seqt: (40, PAIRS*L); rows 0-19 = even batch of the pair, 20-39 = odd
        seqt = np.zeros((2 * D, PAIRS * L), dtype=ml_dtypes.bfloat16)
        wc = np.zeros((2 * D, PAIRS * 2 * HZ), dtype=ml_dtypes.bfloat16)
        for pr in range(PAIRS):
            for k in range(2):
                b = b0 + 2 * pr + k
                seqt[k * D : (k + 1) * D, pr * L : (pr + 1) * L] = seq_il[b].T
                wc[
                    k * D : (k + 1) * D,
                    pr * 2 * HZ + k * HZ : pr * 2 * HZ + (k + 1) * HZ,
                ] = w_all[b]
        in_maps.append({"seqt": seqt, "w": wc})

    LAST_IN_MAPS = in_maps
    nc = _get_nc()
    res = run_bass_kernel_spmd(nc, in_maps, core_ids=list(range(N_CORES)))
    LAST_RESULTS = res

    # device out: (PAIRS*L, 2, HZ) bf16 -> (BS, L, H, D) f32 per core
    outs = []
    for c in range(N_CORES):
        o = res.results[c]["out"].astype(np.float32).reshape(PAIRS, L, 2, HZ)
        outs.append(o.transpose(0, 2, 1, 3).reshape(BS, L, H, D))
    out = np.concatenate(outs, axis=0)
    return np.ascontiguousarray(out.reshape(B, L, H, 1, D))



# revision 3
# speedup vs baseline: 1.0857x; 1.0857x over previous
"""AncProbsLayer Trainium2 kernel (8 NeuronCores, data-parallel over batch).

out[b,l,h,0,z] = sum_d seq[b,l,0,0,d] * P[b,h,d,z]
P[b,h] = diag(1/sqrt_pi_h) V_h diag(exp(lam_h * tau[b,h])) V_h^T diag(sqrt_pi_h)

The GTR eigendecomposition (H=8 independent symmetric 20x20 eigh) and the
per-(b,h) 20x20 transition matrices P are computed on host (tiny, constant
per the reference); the device does the memory-bound part: the
(B*L, 20) x (20, 160) batched matmul producing the 160 MiB output.

Device structure (per core: BS=16 batches = 4 quads of 4):
- 4-way PE row tiling: the 4 batches of a quad sit at SBUF partitions
  {0,32,64,96}+[0,20); four K=20 matmuls with tile_position=(32j,0) run
  CONCURRENTLY in the PE array (the PE is clock-gated at 1.2 GHz here, so
  a lone matmul streams only 1 col/cycle; 4-way tiling recovers 4x).
- uint8 output: each output column (b,h,z) gets scale 255/colbound folded
  into the host-side weights; the f32 PSUM result is cast (RNE, saturating)
  to uint8 by the Vector/Scalar engines and DMA'd out at 1 byte/elem.
  Host dequantizes. Quantization rel-err ~2e-3, far under the 2e-2 gate.
- PSUM->SBUF casts are the critical path (f32 PSUM reads are 1 elem/lane/cy):
  grouped as FD=1280 copies (4 banks x 320) split between DVE and ACT
  weighted by their measured per-copy cost.
- Output DMAs grouped 2 blocks (2560B/partition contiguous) mid-stream,
  1 block at the head (start the write stream early) and tail (fast drain),
  triggers split between the Sync and Tensor engine queues.
"""

import sys

sys.path.insert(0, "/opt/trn_rl_repo")
sys.path.insert(0, "/root/.axon_site")

import numpy as np


def _install_axon_hooks_shim():
    """The agent image's antenv lacks axon_hooks; bass_utils imports it when
    BASS_TRACE=1. Provide it (registering the ctypes NTFF hook if possible)
    so tracing degrades gracefully instead of crashing."""
    try:
        import antenv.axon_hooks  # noqa: F401

        return
    except ImportError:
        pass
    try:
        import types

        mod = types.ModuleType("antenv.axon_hooks")
        _h = [None]
        mod.set_axon_ntff_profile_hook = lambda h: _h.__setitem__(0, h)
        mod.get_axon_ntff_profile_hook = lambda: _h[0]
        sys.modules["antenv.axon_hooks"] = mod
        import antenv

        antenv.axon_hooks = mod
        try:
            from trn_agent_boot.trn_boot import _ntff_profile_via_ctypes

            mod.set_axon_ntff_profile_hook(
                _ntff_profile_via_ctypes("/opt/axon/libaxon_pjrt.so")
            )
        except Exception:
            pass
    except Exception:
        pass


_install_axon_hooks_shim()

B, L, H, D = 128, 2048, 8, 20
N_CORES = 8
BS = B // N_CORES  # batches per core
NQ = BS // 4  # quads per core
HZ = H * D  # 160 output cols per (b, l)
LT = L // 128  # l-tiles per batch
NBLK = NQ * (LT // 2)  # copy blocks: 2 l-tiles x 4 batches each = 1280 cols

_NC = None  # compiled Bass graph cache
LAST_RESULTS = None  # BassKernelResults of the most recent run (for profiling)
LAST_IN_MAPS = None  # per-core input shards of the most recent run

# out-DMA grouping (in blocks): small at head (start write stream early)
# and tail (fast drain), 2-block groups in the middle
DMA_GROUPS = [1, 1] + [2] * 14 + [1, 1]
assert sum(DMA_GROUPS) == NBLK


def _build_nc():
    import concourse.bacc as bacc
    import concourse.tile as tile
    import concourse.mybir as mybir

    f32 = mybir.dt.float32
    bf16 = mybir.dt.bfloat16
    u8 = mybir.dt.uint8
    nc = bacc.Bacc(None, target_bir_lowering=False)

    # member j of each quad lives at DRAM rows 20j..20j+19
    seqs = nc.declare_dram_parameter("seqs", [4 * D, NQ * L], bf16, isOutput=False)
    w = nc.declare_dram_parameter("w", [4 * D, NQ * HZ], bf16, isOutput=False)
    out = nc.declare_dram_parameter("out", [128, NBLK * 1280], u8, isOutput=True)

    with tile.TileContext(nc) as tc:
        with (
            tc.tile_pool(name="spool", bufs=1) as spool,
            tc.tile_pool(name="pp", bufs=1, space="PSUM") as pp,
            tc.tile_pool(name="opool", bufs=4) as opool,
        ):
            st = spool.tile([128, NQ * L], bf16)
            wt = spool.tile([128, NQ * HZ], bf16)
            # quad 0 of seq first so the matmuls start ASAP, then the rest;
            # w interleaved so the first block isn't gated on the whole load
            for j in range(4):
                nc.sync.dma_start(st[32 * j : 32 * j + D, 0:L], seqs[20 * j : 20 * j + D, 0:L])
            for j in range(4):
                nc.sync.dma_start(wt[32 * j : 32 * j + D, :], w[20 * j : 20 * j + D, :])
            for j in range(4):
                nc.sync.dma_start(
                    st[32 * j : 32 * j + D, L:], seqs[20 * j : 20 * j + D, L:]
                )

            ps0 = pp.tile([128, 4, 512], f32)
            ps1 = pp.tile([128, 4, 512], f32)
            pss = [ps0, ps1]

            # engine split for the PSUM->SBUF cast copies (measured ns/copy)
            DVE_COST, ACT_COST = 1460, 1330
            t_dve = t_act = 0

            ogroups = []  # (otile, nblk_in_group, first_blk)
            gi = 0
            blk_in_g = 0
            ot = None

            for blk in range(NBLK):
                q, tp = divmod(blk, LT // 2)
                pt = pss[blk % 2]
                for r in range(2):
                    t = 2 * tp + r
                    for j in range(4):
                        nc.tensor.matmul(
                            pt[:, j, r * HZ : (r + 1) * HZ],
                            st[32 * j : 32 * j + D, q * L + t * 128 : q * L + (t + 1) * 128],
                            wt[32 * j : 32 * j + D, q * HZ : (q + 1) * HZ],
                            start=True,
                            stop=True,
                            tile_position=(32 * j, 0),
                        )
                if blk_in_g == 0:
                    glen = DMA_GROUPS[gi]
                    ot = opool.tile([128, glen, 1280], u8, tag="ob")
                dst = ot[:, blk_in_g, :].rearrange("p (a b) -> p a b", a=4)
                src = pt[:, :, 0 : 2 * HZ]
                if t_dve + DVE_COST <= t_act + ACT_COST:
                    nc.vector.tensor_copy(dst, src)
                    t_dve += DVE_COST
                else:
                    nc.scalar.copy(dst, src)
                    t_act += ACT_COST
                blk_in_g += 1
                if blk_in_g == DMA_GROUPS[gi]:
                    first = blk - blk_in_g + 1
                    dst_d = out.rearrange("p (g x) -> p g x", x=1280)[
                        :, first : first + blk_in_g, :
                    ]
                    nc.sync.dma_start(dst_d, ot[:])
                    gi += 1
                    blk_in_g = 0
    nc.compile()
    return nc


def _get_nc():
    global _NC
    if _NC is None:
        _NC = _build_nc()
    return _NC


def _host_precompute(rate_indices, tau_kernel, exchangeability_kernel, equilibrium_kernel):
    """Everything up to the per-(b,h) 20x20 transition matrices, in float64.

    Returns (wf, scale): wf[b, d, hz] = P[b,h,d,z] * 255/colbound[b,h,z],
    scale[b, hz] = colbound[b,h,z]/255 (the host-side dequant factor).
    """
    ek = exchangeability_kernel.astype(np.float64)[:, 0]  # (H, D, D)
    eq = equilibrium_kernel.astype(np.float64)[:, 0]  # (H, D)

    R = np.logaddexp(ek, 0.0)
    R = 0.5 * (R + R.transpose(0, 2, 1))
    m = eq.max(axis=-1, keepdims=True)
    p = np.exp(eq - m)
    p /= p.sum(axis=-1, keepdims=True)

    Q = R * p[:, None, :]
    diag = Q.sum(axis=-1)
    Q = Q - diag[:, :, None] * np.eye(D)
    mue = (p * diag).sum(axis=-1)
    Q = Q / np.maximum(mue, 1e-16)[:, None, None]

    sq = np.sqrt(p)
    isq = 1.0 / sq
    S = sq[:, :, None] * Q * isq[:, None, :]
    S = 0.5 * (S + S.transpose(0, 2, 1))
    lam, V = np.linalg.eigh(S)  # (H, D), (H, D, D)

    W1 = isq[:, :, None] * V  # (H, D, D)
    W2 = V.transpose(0, 2, 1) * sq[:, None, :]  # (H, D, D)

    tau_g = tau_kernel[rate_indices, np.arange(H)[None, :], 0].astype(np.float64)
    tau = np.logaddexp(np.clip(tau_g, -80.0, 80.0), 0.0)  # (B, H)
    e = np.exp(lam[None, :, :] * tau[:, :, None])  # (B, H, D)

    P = np.einsum("hdk,bhk,hkz->bhdz", W1, e, W2)  # (B, H, D, D)

    # per-column uint8 scale: seq in [0,1) => out_z < sum_d max(P_dz, 0)
    colbound = np.maximum(np.clip(P, 0, None).sum(axis=2), 1e-6)  # (B, H, D)
    s = 255.0 / colbound  # (B, H, Z)
    Ps = P * s[:, :, None, :]
    wf = np.ascontiguousarray(Ps.transpose(0, 2, 1, 3)).reshape(B, D, HZ)
    scale = (colbound / 255.0).reshape(B, HZ).astype(np.float32)
    return wf.astype(np.float32), scale


def kernel(sequences, rate_indices, tau_kernel, exchangeability_kernel, equilibrium_kernel):
    global LAST_RESULTS, LAST_IN_MAPS
    from concourse.bass_utils import run_bass_kernel_spmd
    import ml_dtypes

    sequences = np.asarray(sequences)
    rate_indices = np.asarray(rate_indices)
    tau_kernel = np.asarray(tau_kernel)
    exchangeability_kernel = np.asarray(exchangeability_kernel)
    equilibrium_kernel = np.asarray(equilibrium_kernel)

    wf, scale = _host_precompute(
        rate_indices, tau_kernel, exchangeability_kernel, equilibrium_kernel
    )
    seq = np.asarray(sequences, dtype=np.float32).reshape(B, L, D)

    in_maps = []
    for c in range(N_CORES):
        seqs = np.empty((4 * D, NQ * L), dtype=ml_dtypes.bfloat16)
        wc = np.empty((4 * D, NQ * HZ), dtype=ml_dtypes.bfloat16)
        for q in range(NQ):
            for j in range(4):
                b = c * BS + q * 4 + j
                seqs[20 * j : 20 * j + D, q * L : (q + 1) * L] = seq[b].T
                wc[20 * j : 20 * j + D, q * HZ : (q + 1) * HZ] = wf[b]
        in_maps.append({"seqs": seqs, "w": wc})

    LAST_IN_MAPS = in_maps
    nc = _get_nc()
    res = run_bass_kernel_spmd(nc, in_maps, core_ids=list(range(N_CORES)))
    LAST_RESULTS = res

    # decode: out[p, blk, j, half, hz]; blk=(q,tp); l = (2*tp+half)*128 + p
    outs = []
    for c in range(N_CORES):
        a = res.results[c]["out"].reshape(128, NQ, LT // 2, 4, 2, HZ)
        a = a.transpose(1, 3, 2, 4, 0, 5).reshape(BS, L, HZ).astype(np.float32)
        a *= scale[c * BS : (c + 1) * BS, None, :]
        outs.append(a)
    out = np.concatenate(outs, axis=0)
    return np.ascontiguousarray(out.reshape(B, L, H, 1, D))
